# revision 3
# baseline (speedup 1.0000x reference)
"""GCNConv-with-constraint kernel v2 for 8 Trainium2 NeuronCores.

Same math as the baseline kernel (dst-sharded nodes, edge one-hot scatter via
PE matmuls, on-device W renorm + linear), restructured for wider accumulation:

  - 512-wide destination superblocks: each PSUM agg tile is [128ch, 512dst]
    (one full bank); per-tile scatter matmuls accumulate into a 512-col bank,
    so accumulation chains are ~4x longer and block epilogues (PSUM->SBUF copy,
    W matmul, bias, out-DMA) run once per 512 destinations instead of 128.
  - sel one-hots built in chunks of SEL_CHUNK tiles as [128e, ck, 512] fp16.
  - gather knobs: single_packet, merged call cap.
"""

import math
import os
from contextlib import ExitStack

import numpy as np

import concourse.bass as bass
import concourse.tile as tile
from concourse import bacc, mybir
from concourse.bass_utils import run_bass_kernel_spmd

N_CORES = 8
C = 128  # in/out channels
P = 128  # partitions
SB = 512  # destination superblock width
HALF = 32768  # int16-addressable rows per gather table half
TILES_PER_CALL = 64  # max tiles per dma_gather call
SEL_CHUNK = 16  # tiles per sel-build DVE op pair
SINGLE_PACKET = False

f16 = mybir.dt.float16
f32 = mybir.dt.float32
i16 = mybir.dt.int16

LAST_RESULTS = None


def _prep(x, edge_index, W, b):
    N = x.shape[0]
    npc = math.ceil(N / N_CORES)
    n_sb = math.ceil(npc / SB)

    src = np.asarray(edge_index[0], dtype=np.int64)
    dst = np.asarray(edge_index[1], dtype=np.int64)

    deg = np.bincount(dst, minlength=N).astype(np.float64) + 1.0
    dinv = 1.0 / np.sqrt(deg)
    norm = dinv[src] * dinv[dst]

    ar = np.arange(N, dtype=np.int64)
    src_all = np.concatenate([src, ar])
    dst_all = np.concatenate([dst, ar])
    norm_all = np.concatenate([norm, dinv * dinv]).astype(np.float32)

    shard = dst_all // npc
    dst_loc = dst_all - shard * npc
    sb = dst_loc // SB
    dst_sub = dst_loc - sb * SB  # 0..511
    ishi = (src_all >= HALF).astype(np.int64)
    key = sb * 2 + ishi
    nk = n_sb * 2

    cnt_sk = np.zeros((N_CORES, nk), dtype=np.int64)
    for s in range(N_CORES):
        m = shard == s
        cnt_sk[s] = np.bincount(key[m], minlength=nk)
    tiles_k = (cnt_sk.max(axis=0) + P - 1) // P

    stream_keys = []
    runs = []
    for bb in range(n_sb):
        for half in (0, 1):
            kkey = 2 * bb + half
            if tiles_k[kkey] > 0:
                runs.append([kkey])
                stream_keys.append(kkey)
    stream_pos = np.zeros(nk, dtype=np.int64)
    t = 0
    for kkey in stream_keys:
        stream_pos[kkey] = t
        t += tiles_k[kkey]
    n_tiles = int(t)

    sb_of = np.zeros(n_tiles, dtype=np.int64)
    half_of = np.zeros(n_tiles, dtype=np.int64)
    for kkey in stream_keys:
        t0, tn = stream_pos[kkey], tiles_k[kkey]
        sb_of[t0 : t0 + tn] = kkey // 2
        half_of[t0 : t0 + tn] = kkey % 2
    first_of = np.zeros(n_tiles, dtype=bool)
    last_of = np.zeros(n_tiles, dtype=bool)
    for bb in range(n_sb):
        ts = np.where(sb_of == bb)[0]
        assert len(ts) > 0
        first_of[ts.min()] = True
        last_of[ts.max()] = True

    # gather calls: split each (sb, half) run into chunks of <= TILES_PER_CALL
    calls = []
    for r in runs:
        kkey = r[0]
        t0 = int(stream_pos[kkey])
        L = int(tiles_k[kkey])
        nch = math.ceil(L / TILES_PER_CALL)
        sizes = [L // nch + (1 if i < L % nch else 0) for i in range(nch)]
        o = t0
        for sz in sizes:
            calls.append((o, sz, kkey % 2))
            o += sz

    xtab = np.ascontiguousarray(np.asarray(x).astype(np.float16))
    iota_arr = np.ascontiguousarray(
        np.broadcast_to(
            np.tile(np.arange(SB, dtype=np.float16), SEL_CHUNK), (P, SEL_CHUNK * SB)
        )
    )
    ident = np.eye(C, dtype=np.float32)
    wf = np.ascontiguousarray(np.asarray(W, dtype=np.float32))
    bvec = np.ascontiguousarray(np.asarray(b, dtype=np.float32).reshape(C, 1))

    pad_off = stream_pos * P

    in_maps = []
    for s in range(N_CORES):
        m = shard == s
        sl_src = src_all[m] - HALF * ishi[m]
        sl_ds = dst_sub[m]
        sl_key = key[m]
        sl_nm = norm_all[m]
        order = np.lexsort((sl_src, sl_key))  # bucket-major, src-sorted within
        cnt = np.bincount(sl_key, minlength=nk)
        starts = np.concatenate([[0], np.cumsum(cnt)[:-1]])
        pos_in_grp = np.arange(len(sl_src)) - np.repeat(starts, cnt)
        dst_pos = np.repeat(pad_off, cnt) + pos_in_grp

        S = np.zeros(n_tiles * P, np.int16)
        D = np.full(n_tiles * P, -1.0, np.float16)
        NM = np.zeros(n_tiles * P, np.float16)
        S[dst_pos] = sl_src[order].astype(np.int16)
        D[dst_pos] = sl_ds[order].astype(np.float16)
        NM[dst_pos] = sl_nm[order].astype(np.float16)

        srcw16 = S.reshape(n_tiles * 8, 16).T
        srcw = np.ascontiguousarray(np.tile(srcw16, (8, 1)))

        in_maps.append(
            {
                "xtab": xtab,
                "srcw": srcw,
                "dsts": np.ascontiguousarray(D.reshape(n_tiles, P).T),
                "nrms": np.ascontiguousarray(NM.reshape(n_tiles, P).T),
                "w": wf,
                "bvec": bvec,
                "iota": iota_arr,
                "ident": ident,
            }
        )

    structure = dict(
        N=N,
        npc=npc,
        n_sb=n_sb,
        n_tiles=n_tiles,
        calls=calls,
        sb_of=sb_of,
        half_of=half_of,
        first_of=first_of,
        last_of=last_of,
    )
    return in_maps, structure


def _build_program(st, repeat=1):
    N, n_tiles, n_sb = st["N"], st["n_tiles"], st["n_sb"]
    nc = bacc.Bacc(
        "TRN2", target_bir_lowering=False, debug=False, num_devices=N_CORES,
    )

    xtab = nc.dram_tensor("xtab", [N, C], f16, kind="ExternalInput").ap()
    srcw = nc.dram_tensor("srcw", [P, 8 * n_tiles], i16, kind="ExternalInput").ap()
    dsts = nc.dram_tensor("dsts", [P, n_tiles], f16, kind="ExternalInput").ap()
    nrms = nc.dram_tensor("nrms", [P, n_tiles], f16, kind="ExternalInput").ap()
    w = nc.dram_tensor("w", [C, C], f32, kind="ExternalInput").ap()
    bvec = nc.dram_tensor("bvec", [C, 1], f32, kind="ExternalInput").ap()
    iota = nc.dram_tensor("iota", [P, SEL_CHUNK * SB], f16, kind="ExternalInput").ap()
    ident = nc.dram_tensor("ident", [C, C], f32, kind="ExternalInput").ap()
    outt = nc.dram_tensor("outt", [C, n_sb * SB], f32, kind="ExternalOutput").ap()

    Copy = mybir.ActivationFunctionType.Copy
    Sqrt = mybir.ActivationFunctionType.Sqrt
    Op = mybir.AluOpType

    with tile.TileContext(nc) as tc, ExitStack() as ctx:
        cpool = ctx.enter_context(tc.tile_pool(name="const", bufs=1))
        iota_sb = cpool.tile([P, SEL_CHUNK, SB], f16, tag="iota")
        wnT_sb = cpool.tile([C, C], f32, tag="wnT")
        bias_sb = cpool.tile([C, 1], f32, tag="bias")
        nc.sync.dma_start(iota_sb[:], iota[:])
        nc.sync.dma_start(bias_sb[:], bvec[:])

        # ---- prologue: Wn = W * min(1, 1/||W[:,i]||); WnT = Wn^T ----
        with (
            tc.tile_pool(name="prol", bufs=1) as pp,
            tc.tile_pool(name="prol_ps", bufs=1, space="PSUM") as ppp,
        ):
            w_sb = pp.tile([C, C], f32, tag="w")
            nc.sync.dma_start(w_sb[:], w[:])
            ident_sb = pp.tile([C, C], f32, tag="ident")
            nc.sync.dma_start(ident_sb[:], ident[:])
            wsq = pp.tile([C, C], f32, tag="wsq")
            nc.vector.tensor_tensor(out=wsq[:], in0=w_sb[:], in1=w_sb[:], op=Op.mult)
            ones_c = pp.tile([C, 1], f32, tag="ones_c")
            nc.vector.memset(ones_c[:], 1.0)
            cn_ps = ppp.tile([1, C], f32, tag="cn")
            nc.tensor.matmul(cn_ps[:], lhsT=ones_c[:], rhs=wsq[:], start=True, stop=True)
            nrm_sb = pp.tile([1, C], f32, tag="nrm")
            nc.scalar.activation(nrm_sb[:], cn_ps[:], Sqrt)
            rec_sb = pp.tile([1, C], f32, tag="rec")
            nc.vector.reciprocal(rec_sb[:], nrm_sb[:])
            scl_sb = pp.tile([1, C], f32, tag="scl")
            nc.vector.tensor_scalar(
                out=scl_sb[:], in0=rec_sb[:], scalar1=1.0, scalar2=None, op0=Op.min
            )
            ones_r = pp.tile([1, C], f32, tag="ones_r")
            nc.vector.memset(ones_r[:], 1.0)
            sbc_ps = ppp.tile([C, C], f32, tag="sbc")
            nc.tensor.matmul(
                sbc_ps[:], lhsT=ones_r[:], rhs=scl_sb[:], start=True, stop=True
            )
            wn_sb = pp.tile([C, C], f32, tag="wn")
            nc.vector.tensor_tensor(out=wn_sb[:], in0=w_sb[:], in1=sbc_ps[:], op=Op.mult)
            wnT_ps = ppp.tile([C, C], f32, tag="wnT_ps")
            nc.tensor.matmul(
                wnT_ps[:], lhsT=wn_sb[:], rhs=ident_sb[:], start=True, stop=True
            )
            nc.scalar.activation(wnT_sb[:], wnT_ps[:], Copy)

        # ---- edge phase ----
        mpool = ctx.enter_context(tc.tile_pool(name="meta", bufs=3))
        gpool = ctx.enter_context(tc.tile_pool(name="gather", bufs=3))
        spool = ctx.enter_context(tc.tile_pool(name="sel", bufs=3))
        apool = ctx.enter_context(tc.tile_pool(name="aggsb", bufs=2))
        opool = ctx.enter_context(tc.tile_pool(name="outsb", bufs=2))
        agg_psp = ctx.enter_context(tc.tile_pool(name="aggps", bufs=2, space="PSUM"))
        out_psp = ctx.enter_context(tc.tile_pool(name="outps", bufs=2, space="PSUM"))

        xtab_hi = xtab[HALF:, :] if N > HALF else None
        sb_of, first_of, last_of = st["sb_of"], st["first_of"], st["last_of"]
        for _rep in range(repeat):
            agg_ps = None
            for ci, (t0, kg, half) in enumerate(st["calls"]):
                src_sl = mpool.tile([P, kg * 8], i16, tag="srcsl")
                nc.sync.dma_start(src_sl[:], srcw[:, 8 * t0 : 8 * (t0 + kg)])
                dst_sl = mpool.tile([P, kg], f16, tag="dstsl")
                nc.sync.dma_start(dst_sl[:], dsts[:, t0 : t0 + kg])
                nrm_sl = mpool.tile([P, kg], f16, tag="nrmsl")
                nc.sync.dma_start(nrm_sl[:], nrms[:, t0 : t0 + kg])
                gbuf = gpool.tile([P, TILES_PER_CALL, C], f16, tag="gbuf")
                nc.gpsimd.dma_gather(
                    out_ap=gbuf[:, :kg, :],
                    in_ap=(xtab[:] if half == 0 else xtab_hi),
                    idxs_ap=src_sl[:],
                    num_idxs=kg * P,
                    num_idxs_reg=kg * P,
                    elem_size=C,
                    single_packet=SINGLE_PACKET,
                )
                for c0 in range(0, kg, SEL_CHUNK):
                    ck = min(SEL_CHUNK, kg - c0)
                    sel3 = spool.tile([P, SEL_CHUNK, SB], f16, tag="sel")
                    nc.vector.tensor_tensor(
                        out=sel3[:, :ck, :],
                        in0=iota_sb[:, :ck, :],
                        in1=dst_sl[:, c0 : c0 + ck]
                        .unsqueeze(2)
                        .to_broadcast([P, ck, SB]),
                        op=Op.is_equal,
                    )
                    nc.vector.tensor_tensor(
                        out=sel3[:, :ck, :],
                        in0=sel3[:, :ck, :],
                        in1=nrm_sl[:, c0 : c0 + ck]
                        .unsqueeze(2)
                        .to_broadcast([P, ck, SB]),
                        op=Op.mult,
                    )
                    for slot in range(c0, c0 + ck):
                        t = t0 + slot
                        bb = int(sb_of[t])
                        if first_of[t]:
                            agg_ps = agg_psp.tile(
                                [C, SB], f32, tag="aggps", name=f"aggps_b{bb}"
                            )
                        nc.tensor.matmul(
                            agg_ps[:],
                            lhsT=gbuf[:, slot, :],
                            rhs=sel3[:, slot - c0, :],
                            start=bool(first_of[t]),
                            stop=bool(last_of[t]),
                        )
                        if last_of[t]:
                            agg_sb = apool.tile([C, SB], f32, tag="aggsb")
                            nc.scalar.activation(agg_sb[:], agg_ps[:], Copy)
                            outT_ps = out_psp.tile([C, SB], f32, tag="outps")
                            nc.tensor.matmul(
                                outT_ps[:],
                                lhsT=wnT_sb[:],
                                rhs=agg_sb[:],
                                start=True,
                                stop=True,
                            )
                            outT_sb = opool.tile([C, SB], f32, tag="outsb")
                            nc.vector.tensor_scalar(
                                out=outT_sb[:],
                                in0=outT_ps[:],
                                scalar1=bias_sb[:],
                                scalar2=None,
                                op0=Op.add,
                            )
                            nc.sync.dma_start(
                                outt[:, bb * SB : (bb + 1) * SB], outT_sb[:]
                            )

    nc.compile()
    return nc


def kernel(x, edge_index, W, b):
    global LAST_RESULTS
    x = np.asarray(x)
    N = x.shape[0]
    assert x.shape[1] == C and W.shape == (C, C)

    in_maps, st = _prep(x, edge_index, W, b)
    nc = _build_program(st)

    os.environ.setdefault("BASS_NEVER_TRACE", "1")
    res = run_bass_kernel_spmd(nc, in_maps, list(range(N_CORES)))
    LAST_RESULTS = res

    npc = st["npc"]
    shards = []
    for s in range(N_CORES):
        lo = s * npc
        hi = min((s + 1) * npc, N)
        outt = res.results[s]["outt"]
        shards.append(outt[:, : hi - lo].T)
    return np.ascontiguousarray(np.concatenate(shards, axis=0), dtype=np.float32)


# revision 4
# speedup vs baseline: 1.1360x; 1.1360x over previous
"""GCNConv-with-constraint kernel v2 for 8 Trainium2 NeuronCores.

Same math as the baseline kernel (dst-sharded nodes, edge one-hot scatter via
PE matmuls, on-device W renorm + linear), restructured for wider accumulation:

  - 512-wide destination superblocks: each PSUM agg tile is [128ch, 512dst]
    (one full bank); per-tile scatter matmuls accumulate into a 512-col bank,
    so accumulation chains are ~4x longer and block epilogues (PSUM->SBUF copy,
    W matmul, bias, out-DMA) run once per 512 destinations instead of 128.
  - sel one-hots built in chunks of SEL_CHUNK tiles as [128e, ck, 512] fp16.
  - gather knobs: single_packet, merged call cap.
"""

import math
import os
from contextlib import ExitStack

import numpy as np

import concourse.bass as bass
import concourse.tile as tile
from concourse import bacc, mybir
from concourse.bass_utils import run_bass_kernel_spmd

N_CORES = 8
C = 128  # in/out channels
P = 128  # partitions
SB = 512  # destination superblock width
HALF = 32768  # int16-addressable rows per gather table half
TILES_PER_CALL = 64  # max tiles per dma_gather call
SEL_CHUNK = 17  # tiles per sel-build DVE op pair (33-tile calls -> 2 chunks)
SINGLE_PACKET = False

f16 = mybir.dt.float16
f32 = mybir.dt.float32
i16 = mybir.dt.int16

LAST_RESULTS = None


def _prep(x, edge_index, W, b):
    N = x.shape[0]
    npc = math.ceil(N / N_CORES)
    n_sb = math.ceil(npc / SB)

    src = np.asarray(edge_index[0], dtype=np.int64)
    dst = np.asarray(edge_index[1], dtype=np.int64)

    deg = np.bincount(dst, minlength=N).astype(np.float64) + 1.0
    dinv = 1.0 / np.sqrt(deg)
    norm = dinv[src] * dinv[dst]

    ar = np.arange(N, dtype=np.int64)
    src_all = np.concatenate([src, ar])
    dst_all = np.concatenate([dst, ar])
    norm_all = np.concatenate([norm, dinv * dinv]).astype(np.float32)

    shard = dst_all // npc
    dst_loc = dst_all - shard * npc
    sb = dst_loc // SB
    dst_sub = dst_loc - sb * SB  # 0..511
    ishi = (src_all >= HALF).astype(np.int64)
    key = sb * 2 + ishi
    nk = n_sb * 2

    cnt_sk = np.zeros((N_CORES, nk), dtype=np.int64)
    for s in range(N_CORES):
        m = shard == s
        cnt_sk[s] = np.bincount(key[m], minlength=nk)
    tiles_k = (cnt_sk.max(axis=0) + P - 1) // P

    stream_keys = []
    runs = []
    for bb in range(n_sb):
        for half in (0, 1):
            kkey = 2 * bb + half
            if tiles_k[kkey] > 0:
                runs.append([kkey])
                stream_keys.append(kkey)
    stream_pos = np.zeros(nk, dtype=np.int64)
    t = 0
    for kkey in stream_keys:
        stream_pos[kkey] = t
        t += tiles_k[kkey]
    n_tiles = int(t)

    sb_of = np.zeros(n_tiles, dtype=np.int64)
    half_of = np.zeros(n_tiles, dtype=np.int64)
    for kkey in stream_keys:
        t0, tn = stream_pos[kkey], tiles_k[kkey]
        sb_of[t0 : t0 + tn] = kkey // 2
        half_of[t0 : t0 + tn] = kkey % 2
    first_of = np.zeros(n_tiles, dtype=bool)
    last_of = np.zeros(n_tiles, dtype=bool)
    for bb in range(n_sb):
        ts = np.where(sb_of == bb)[0]
        assert len(ts) > 0
        first_of[ts.min()] = True
        last_of[ts.max()] = True

    # gather calls: split each (sb, half) run into chunks of <= TILES_PER_CALL
    calls = []
    for r in runs:
        kkey = r[0]
        t0 = int(stream_pos[kkey])
        L = int(tiles_k[kkey])
        nch = math.ceil(L / TILES_PER_CALL)
        sizes = [L // nch + (1 if i < L % nch else 0) for i in range(nch)]
        o = t0
        for sz in sizes:
            calls.append((o, sz, kkey % 2))
            o += sz

    xtab = np.ascontiguousarray(np.asarray(x).astype(np.float16))
    iota_arr = np.ascontiguousarray(
        np.broadcast_to(
            np.tile(np.arange(SB, dtype=np.float16), SEL_CHUNK), (P, SEL_CHUNK * SB)
        )
    )
    ident = np.eye(C, dtype=np.float32)
    wf = np.ascontiguousarray(np.asarray(W, dtype=np.float32))
    bvec = np.ascontiguousarray(np.asarray(b, dtype=np.float32).reshape(C, 1))

    pad_off = stream_pos * P

    in_maps = []
    for s in range(N_CORES):
        m = shard == s
        sl_src = src_all[m] - HALF * ishi[m]
        sl_ds = dst_sub[m]
        sl_key = key[m]
        sl_nm = norm_all[m]
        order = np.lexsort((sl_src, sl_key))  # bucket-major, src-sorted within
        cnt = np.bincount(sl_key, minlength=nk)
        starts = np.concatenate([[0], np.cumsum(cnt)[:-1]])
        pos_in_grp = np.arange(len(sl_src)) - np.repeat(starts, cnt)
        dst_pos = np.repeat(pad_off, cnt) + pos_in_grp

        S = np.zeros(n_tiles * P, np.int16)
        D = np.full(n_tiles * P, -1.0, np.float16)
        NM = np.zeros(n_tiles * P, np.float16)
        S[dst_pos] = sl_src[order].astype(np.int16)
        D[dst_pos] = sl_ds[order].astype(np.float16)
        NM[dst_pos] = sl_nm[order].astype(np.float16)

        srcw16 = S.reshape(n_tiles * 8, 16).T
        srcw = np.ascontiguousarray(np.tile(srcw16, (8, 1)))

        in_maps.append(
            {
                "xtab": xtab,
                "srcw": srcw,
                "dsts": np.ascontiguousarray(D.reshape(n_tiles, P).T),
                "nrms": np.ascontiguousarray(NM.reshape(n_tiles, P).T),
                "w": wf,
                "bvec": bvec,
                "iota": iota_arr,
                "ident": ident,
            }
        )

    structure = dict(
        N=N,
        npc=npc,
        n_sb=n_sb,
        n_tiles=n_tiles,
        calls=calls,
        sb_of=sb_of,
        half_of=half_of,
        first_of=first_of,
        last_of=last_of,
    )
    return in_maps, structure


def _build_program(st, repeat=1):
    N, n_tiles, n_sb = st["N"], st["n_tiles"], st["n_sb"]
    nc = bacc.Bacc(
        "TRN2", target_bir_lowering=False, debug=False, num_devices=N_CORES,
    )

    xtab = nc.dram_tensor("xtab", [N, C], f16, kind="ExternalInput").ap()
    srcw = nc.dram_tensor("srcw", [P, 8 * n_tiles], i16, kind="ExternalInput").ap()
    dsts = nc.dram_tensor("dsts", [P, n_tiles], f16, kind="ExternalInput").ap()
    nrms = nc.dram_tensor("nrms", [P, n_tiles], f16, kind="ExternalInput").ap()
    w = nc.dram_tensor("w", [C, C], f32, kind="ExternalInput").ap()
    bvec = nc.dram_tensor("bvec", [C, 1], f32, kind="ExternalInput").ap()
    iota = nc.dram_tensor("iota", [P, SEL_CHUNK * SB], f16, kind="ExternalInput").ap()
    ident = nc.dram_tensor("ident", [C, C], f32, kind="ExternalInput").ap()
    outt = nc.dram_tensor("outt", [C, n_sb * SB], f32, kind="ExternalOutput").ap()

    Copy = mybir.ActivationFunctionType.Copy
    Sqrt = mybir.ActivationFunctionType.Sqrt
    Op = mybir.AluOpType

    with tile.TileContext(nc) as tc, ExitStack() as ctx:
        cpool = ctx.enter_context(tc.tile_pool(name="const", bufs=1))
        iota_sb = cpool.tile([P, SEL_CHUNK, SB], f16, tag="iota")
        wnT_sb = cpool.tile([C, C], f32, tag="wnT")
        bias_sb = cpool.tile([C, 1], f32, tag="bias")
        nc.sync.dma_start(iota_sb[:], iota[:])
        nc.sync.dma_start(bias_sb[:], bvec[:])

        # ---- prologue: Wn = W * min(1, 1/||W[:,i]||); WnT = Wn^T ----
        with (
            tc.tile_pool(name="prol", bufs=1) as pp,
            tc.tile_pool(name="prol_ps", bufs=1, space="PSUM") as ppp,
        ):
            w_sb = pp.tile([C, C], f32, tag="w")
            nc.sync.dma_start(w_sb[:], w[:])
            ident_sb = pp.tile([C, C], f32, tag="ident")
            nc.sync.dma_start(ident_sb[:], ident[:])
            wsq = pp.tile([C, C], f32, tag="wsq")
            nc.vector.tensor_tensor(out=wsq[:], in0=w_sb[:], in1=w_sb[:], op=Op.mult)
            ones_c = pp.tile([C, 1], f32, tag="ones_c")
            nc.vector.memset(ones_c[:], 1.0)
            cn_ps = ppp.tile([1, C], f32, tag="cn")
            nc.tensor.matmul(cn_ps[:], lhsT=ones_c[:], rhs=wsq[:], start=True, stop=True)
            nrm_sb = pp.tile([1, C], f32, tag="nrm")
            nc.scalar.activation(nrm_sb[:], cn_ps[:], Sqrt)
            rec_sb = pp.tile([1, C], f32, tag="rec")
            nc.vector.reciprocal(rec_sb[:], nrm_sb[:])
            scl_sb = pp.tile([1, C], f32, tag="scl")
            nc.vector.tensor_scalar(
                out=scl_sb[:], in0=rec_sb[:], scalar1=1.0, scalar2=None, op0=Op.min
            )
            ones_r = pp.tile([1, C], f32, tag="ones_r")
            nc.vector.memset(ones_r[:], 1.0)
            sbc_ps = ppp.tile([C, C], f32, tag="sbc")
            nc.tensor.matmul(
                sbc_ps[:], lhsT=ones_r[:], rhs=scl_sb[:], start=True, stop=True
            )
            wn_sb = pp.tile([C, C], f32, tag="wn")
            nc.vector.tensor_tensor(out=wn_sb[:], in0=w_sb[:], in1=sbc_ps[:], op=Op.mult)
            wnT_ps = ppp.tile([C, C], f32, tag="wnT_ps")
            nc.tensor.matmul(
                wnT_ps[:], lhsT=wn_sb[:], rhs=ident_sb[:], start=True, stop=True
            )
            nc.scalar.activation(wnT_sb[:], wnT_ps[:], Copy)

        # ---- edge phase ----
        mpool = ctx.enter_context(tc.tile_pool(name="meta", bufs=4))
        gpool = ctx.enter_context(tc.tile_pool(name="gather", bufs=4))
        spool = ctx.enter_context(tc.tile_pool(name="sel", bufs=3))
        apool = ctx.enter_context(tc.tile_pool(name="aggsb", bufs=2))
        opool = ctx.enter_context(tc.tile_pool(name="outsb", bufs=2))
        agg_psp = ctx.enter_context(tc.tile_pool(name="aggps", bufs=2, space="PSUM"))
        out_psp = ctx.enter_context(tc.tile_pool(name="outps", bufs=2, space="PSUM"))

        xtab_hi = xtab[HALF:, :] if N > HALF else None
        sb_of, first_of, last_of = st["sb_of"], st["first_of"], st["last_of"]
        for _rep in range(repeat):
            agg_ps = None
            for ci, (t0, kg, half) in enumerate(st["calls"]):
                src_sl = mpool.tile([P, kg * 8], i16, tag="srcsl")
                nc.sync.dma_start(src_sl[:], srcw[:, 8 * t0 : 8 * (t0 + kg)])
                dst_sl = mpool.tile([P, kg], f16, tag="dstsl")
                nc.sync.dma_start(dst_sl[:], dsts[:, t0 : t0 + kg])
                nrm_sl = mpool.tile([P, kg], f16, tag="nrmsl")
                nc.sync.dma_start(nrm_sl[:], nrms[:, t0 : t0 + kg])
                gbuf = gpool.tile([P, TILES_PER_CALL, C], f16, tag="gbuf")
                nc.gpsimd.dma_gather(
                    out_ap=gbuf[:, :kg, :],
                    in_ap=(xtab[:] if half == 0 else xtab_hi),
                    idxs_ap=src_sl[:],
                    num_idxs=kg * P,
                    num_idxs_reg=kg * P,
                    elem_size=C,
                    single_packet=SINGLE_PACKET,
                )
                for c0 in range(0, kg, SEL_CHUNK):
                    ck = min(SEL_CHUNK, kg - c0)
                    sel3 = spool.tile([P, SEL_CHUNK, SB], f16, tag="sel")
                    nc.vector.tensor_tensor(
                        out=sel3[:, :ck, :],
                        in0=iota_sb[:, :ck, :],
                        in1=dst_sl[:, c0 : c0 + ck]
                        .unsqueeze(2)
                        .to_broadcast([P, ck, SB]),
                        op=Op.is_equal,
                    )
                    nc.vector.tensor_tensor(
                        out=sel3[:, :ck, :],
                        in0=sel3[:, :ck, :],
                        in1=nrm_sl[:, c0 : c0 + ck]
                        .unsqueeze(2)
                        .to_broadcast([P, ck, SB]),
                        op=Op.mult,
                    )
                    for slot in range(c0, c0 + ck):
                        t = t0 + slot
                        bb = int(sb_of[t])
                        if first_of[t]:
                            agg_ps = agg_psp.tile(
                                [C, SB], f32, tag="aggps", name=f"aggps_b{bb}"
                            )
                        nc.tensor.matmul(
                            agg_ps[:],
                            lhsT=gbuf[:, slot, :],
                            rhs=sel3[:, slot - c0, :],
                            start=bool(first_of[t]),
                            stop=bool(last_of[t]),
                        )
                        if last_of[t]:
                            agg_sb = apool.tile([C, SB], f32, tag="aggsb")
                            nc.scalar.activation(agg_sb[:], agg_ps[:], Copy)
                            outT_ps = out_psp.tile([C, SB], f32, tag="outps")
                            nc.tensor.matmul(
                                outT_ps[:],
                                lhsT=wnT_sb[:],
                                rhs=agg_sb[:],
                                start=True,
                                stop=True,
                            )
                            outT_sb = opool.tile([C, SB], f32, tag="outsb")
                            nc.vector.tensor_scalar(
                                out=outT_sb[:],
                                in0=outT_ps[:],
                                scalar1=bias_sb[:],
                                scalar2=None,
                                op0=Op.add,
                            )
                            nc.sync.dma_start(
                                outt[:, bb * SB : (bb + 1) * SB], outT_sb[:]
                            )

    nc.compile()
    return nc


def kernel(x, edge_index, W, b):
    global LAST_RESULTS
    x = np.asarray(x)
    N = x.shape[0]
    assert x.shape[1] == C and W.shape == (C, C)

    in_maps, st = _prep(x, edge_index, W, b)
    nc = _build_program(st)

    os.environ.setdefault("BASS_NEVER_TRACE", "1")
    res = run_bass_kernel_spmd(nc, in_maps, list(range(N_CORES)))
    LAST_RESULTS = res

    npc = st["npc"]
    shards = []
    for s in range(N_CORES):
        lo = s * npc
        hi = min((s + 1) * npc, N)
        outt = res.results[s]["outt"]
        shards.append(outt[:, : hi - lo].T)
    return np.ascontiguousarray(np.concatenate(shards, axis=0), dtype=np.float32)


# revision 5
# speedup vs baseline: 1.6689x; 1.4692x over previous
"""GCNConv-with-constraint kernel v2 for 8 Trainium2 NeuronCores.

Same math as the baseline kernel (dst-sharded nodes, edge one-hot scatter via
PE matmuls, on-device W renorm + linear), restructured for wider accumulation:

  - 512-wide destination superblocks: each PSUM agg tile is [128ch, 512dst]
    (one full bank); per-tile scatter matmuls accumulate into a 512-col bank,
    so accumulation chains are ~4x longer and block epilogues (PSUM->SBUF copy,
    W matmul, bias, out-DMA) run once per 512 destinations instead of 128.
  - sel one-hots built in chunks of SEL_CHUNK tiles as [128e, ck, 512] fp16.
  - gather knobs: single_packet, merged call cap.
"""

import math
import os
from contextlib import ExitStack

import numpy as np

import concourse.bass as bass
import concourse.tile as tile
from concourse import bacc, mybir
from concourse.bass_utils import run_bass_kernel_spmd

N_CORES = 8
C = 128  # in/out channels
P = 128  # partitions
SB = 512  # destination superblock width
HALF = 32768  # int16-addressable rows per gather table half
TILES_PER_CALL = 64  # max tiles per dma_gather call
SEL_CHUNK = 17  # tiles per sel-build DVE op pair (33-tile calls -> 2 chunks)
SINGLE_PACKET = False

f16 = mybir.dt.float16
f32 = mybir.dt.float32
i16 = mybir.dt.int16

LAST_RESULTS = None


def _prep(x, edge_index, W, b):
    N = x.shape[0]
    npc = math.ceil(N / N_CORES)
    n_sb = math.ceil(npc / SB)

    src = np.asarray(edge_index[0], dtype=np.int64)
    dst = np.asarray(edge_index[1], dtype=np.int64)

    deg = np.bincount(dst, minlength=N).astype(np.float64) + 1.0
    dinv = 1.0 / np.sqrt(deg)
    norm = dinv[src] * dinv[dst]

    ar = np.arange(N, dtype=np.int64)
    src_all = np.concatenate([src, ar])
    dst_all = np.concatenate([dst, ar])
    norm_all = np.concatenate([norm, dinv * dinv]).astype(np.float32)

    shard = dst_all // npc
    dst_loc = dst_all - shard * npc
    sb = dst_loc // SB
    dst_sub = dst_loc - sb * SB  # 0..511
    ishi = (src_all >= HALF).astype(np.int64)
    key = sb * 2 + ishi
    nk = n_sb * 2

    cnt_sk = np.zeros((N_CORES, nk), dtype=np.int64)
    for s in range(N_CORES):
        m = shard == s
        cnt_sk[s] = np.bincount(key[m], minlength=nk)
    tiles_k = (cnt_sk.max(axis=0) + P - 1) // P

    stream_keys = []
    runs = []
    for bb in range(n_sb):
        for half in (0, 1):
            kkey = 2 * bb + half
            if tiles_k[kkey] > 0:
                runs.append([kkey])
                stream_keys.append(kkey)
    stream_pos = np.zeros(nk, dtype=np.int64)
    t = 0
    for kkey in stream_keys:
        stream_pos[kkey] = t
        t += tiles_k[kkey]
    n_tiles = int(t)

    sb_of = np.zeros(n_tiles, dtype=np.int64)
    half_of = np.zeros(n_tiles, dtype=np.int64)
    for kkey in stream_keys:
        t0, tn = stream_pos[kkey], tiles_k[kkey]
        sb_of[t0 : t0 + tn] = kkey // 2
        half_of[t0 : t0 + tn] = kkey % 2
    first_of = np.zeros(n_tiles, dtype=bool)
    last_of = np.zeros(n_tiles, dtype=bool)
    for bb in range(n_sb):
        ts = np.where(sb_of == bb)[0]
        assert len(ts) > 0
        first_of[ts.min()] = True
        last_of[ts.max()] = True

    # gather calls: split each (sb, half) run into chunks of <= TILES_PER_CALL
    calls = []
    for r in runs:
        kkey = r[0]
        t0 = int(stream_pos[kkey])
        L = int(tiles_k[kkey])
        nch = math.ceil(L / TILES_PER_CALL)
        sizes = [L // nch + (1 if i < L % nch else 0) for i in range(nch)]
        o = t0
        for sz in sizes:
            calls.append((o, sz, kkey % 2))
            o += sz

    xtab = np.ascontiguousarray(np.asarray(x).astype(np.float16))
    iota_arr = np.ascontiguousarray(
        np.broadcast_to(
            np.tile(np.arange(SB, dtype=np.float16), SEL_CHUNK), (P, SEL_CHUNK * SB)
        )
    )
    ident = np.eye(C, dtype=np.float32)
    wf = np.ascontiguousarray(np.asarray(W, dtype=np.float32))
    bvec = np.ascontiguousarray(np.asarray(b, dtype=np.float32).reshape(C, 1))

    pad_off = stream_pos * P

    in_maps = []
    for s in range(N_CORES):
        m = shard == s
        sl_src = src_all[m] - HALF * ishi[m]
        sl_ds = dst_sub[m]
        sl_key = key[m]
        sl_nm = norm_all[m]
        order = np.lexsort((sl_src, sl_key))  # bucket-major, src-sorted within
        cnt = np.bincount(sl_key, minlength=nk)
        starts = np.concatenate([[0], np.cumsum(cnt)[:-1]])
        pos_in_grp = np.arange(len(sl_src)) - np.repeat(starts, cnt)
        dst_pos = np.repeat(pad_off, cnt) + pos_in_grp

        S = np.zeros(n_tiles * P, np.int16)
        D = np.full(n_tiles * P, -1.0, np.float16)
        NM = np.zeros(n_tiles * P, np.float16)
        S[dst_pos] = sl_src[order].astype(np.int16)
        D[dst_pos] = sl_ds[order].astype(np.float16)
        NM[dst_pos] = sl_nm[order].astype(np.float16)

        srcw16 = S.reshape(n_tiles * 8, 16).T
        srcw = np.ascontiguousarray(np.tile(srcw16, (8, 1)))

        in_maps.append(
            {
                "xtab": xtab,
                "srcw": srcw,
                "dsts": np.ascontiguousarray(D.reshape(n_tiles, P).T),
                "nrms": np.ascontiguousarray(NM.reshape(n_tiles, P).T),
                "w": wf,
                "bvec": bvec,
                "iota": iota_arr,
                "ident": ident,
            }
        )

    structure = dict(
        N=N,
        npc=npc,
        n_sb=n_sb,
        n_tiles=n_tiles,
        calls=calls,
        sb_of=sb_of,
        half_of=half_of,
        first_of=first_of,
        last_of=last_of,
    )
    return in_maps, structure


def _build_program(st, repeat=1):
    N, n_tiles, n_sb = st["N"], st["n_tiles"], st["n_sb"]
    nc = bacc.Bacc(
        "TRN2", target_bir_lowering=False, debug=False, num_devices=N_CORES,
    )

    xtab = nc.dram_tensor("xtab", [N, C], f16, kind="ExternalInput").ap()
    srcw = nc.dram_tensor("srcw", [P, 8 * n_tiles], i16, kind="ExternalInput").ap()
    dsts = nc.dram_tensor("dsts", [P, n_tiles], f16, kind="ExternalInput").ap()
    nrms = nc.dram_tensor("nrms", [P, n_tiles], f16, kind="ExternalInput").ap()
    w = nc.dram_tensor("w", [C, C], f32, kind="ExternalInput").ap()
    bvec = nc.dram_tensor("bvec", [C, 1], f32, kind="ExternalInput").ap()
    iota = nc.dram_tensor("iota", [P, SEL_CHUNK * SB], f16, kind="ExternalInput").ap()
    ident = nc.dram_tensor("ident", [C, C], f32, kind="ExternalInput").ap()
    outt = nc.dram_tensor("outt", [C, n_sb * SB], f32, kind="ExternalOutput").ap()

    Copy = mybir.ActivationFunctionType.Copy
    Sqrt = mybir.ActivationFunctionType.Sqrt
    Op = mybir.AluOpType

    with tile.TileContext(nc) as tc, ExitStack() as ctx:
        cpool = ctx.enter_context(tc.tile_pool(name="const", bufs=1))
        iota_sb = cpool.tile([P, SEL_CHUNK, SB], f16, tag="iota")
        wnT_sb = cpool.tile([C, C], f32, tag="wnT")
        bias_sb = cpool.tile([C, 1], f32, tag="bias")
        nc.sync.dma_start(iota_sb[:], iota[:])
        nc.sync.dma_start(bias_sb[:], bvec[:])
        metap = ctx.enter_context(tc.tile_pool(name="metar", bufs=1))
        srcw_sb = metap.tile([P, 8 * n_tiles], i16, tag="srcwr")
        nc.sync.dma_start(srcw_sb[:], srcw[:])
        dsts_sb = metap.tile([P, n_tiles], f16, tag="dstsr")
        nc.sync.dma_start(dsts_sb[:], dsts[:])
        nrms_sb = metap.tile([P, n_tiles], f16, tag="nrmsr")
        nc.sync.dma_start(nrms_sb[:], nrms[:])

        # ---- prologue: Wn = W * min(1, 1/||W[:,i]||); WnT = Wn^T ----
        with (
            tc.tile_pool(name="prol", bufs=1) as pp,
            tc.tile_pool(name="prol_ps", bufs=1, space="PSUM") as ppp,
        ):
            w_sb = pp.tile([C, C], f32, tag="w")
            nc.sync.dma_start(w_sb[:], w[:])
            ident_sb = pp.tile([C, C], f32, tag="ident")
            nc.sync.dma_start(ident_sb[:], ident[:])
            wsq = pp.tile([C, C], f32, tag="wsq")
            nc.vector.tensor_tensor(out=wsq[:], in0=w_sb[:], in1=w_sb[:], op=Op.mult)
            ones_c = pp.tile([C, 1], f32, tag="ones_c")
            nc.vector.memset(ones_c[:], 1.0)
            cn_ps = ppp.tile([1, C], f32, tag="cn")
            nc.tensor.matmul(cn_ps[:], lhsT=ones_c[:], rhs=wsq[:], start=True, stop=True)
            nrm_sb = pp.tile([1, C], f32, tag="nrm")
            nc.scalar.activation(nrm_sb[:], cn_ps[:], Sqrt)
            rec_sb = pp.tile([1, C], f32, tag="rec")
            nc.vector.reciprocal(rec_sb[:], nrm_sb[:])
            scl_sb = pp.tile([1, C], f32, tag="scl")
            nc.vector.tensor_scalar(
                out=scl_sb[:], in0=rec_sb[:], scalar1=1.0, scalar2=None, op0=Op.min
            )
            ones_r = pp.tile([1, C], f32, tag="ones_r")
            nc.vector.memset(ones_r[:], 1.0)
            sbc_ps = ppp.tile([C, C], f32, tag="sbc")
            nc.tensor.matmul(
                sbc_ps[:], lhsT=ones_r[:], rhs=scl_sb[:], start=True, stop=True
            )
            wn_sb = pp.tile([C, C], f32, tag="wn")
            nc.vector.tensor_tensor(out=wn_sb[:], in0=w_sb[:], in1=sbc_ps[:], op=Op.mult)
            wnT_ps = ppp.tile([C, C], f32, tag="wnT_ps")
            nc.tensor.matmul(
                wnT_ps[:], lhsT=wn_sb[:], rhs=ident_sb[:], start=True, stop=True
            )
            nc.scalar.activation(wnT_sb[:], wnT_ps[:], Copy)

        # ---- edge phase ----
        gpool = ctx.enter_context(tc.tile_pool(name="gather", bufs=4))
        spool = ctx.enter_context(tc.tile_pool(name="sel", bufs=3))
        apool = ctx.enter_context(tc.tile_pool(name="aggsb", bufs=2))
        opool = ctx.enter_context(tc.tile_pool(name="outsb", bufs=2))
        agg_psp = ctx.enter_context(tc.tile_pool(name="aggps", bufs=2, space="PSUM"))
        out_psp = ctx.enter_context(tc.tile_pool(name="outps", bufs=2, space="PSUM"))

        xtab_hi = xtab[HALF:, :] if N > HALF else None
        sb_of, first_of, last_of = st["sb_of"], st["first_of"], st["last_of"]
        for _rep in range(repeat):
            agg_ps = None
            for ci, (t0, kg, half) in enumerate(st["calls"]):
                gbuf = gpool.tile([P, TILES_PER_CALL, C], f16, tag="gbuf")
                nc.gpsimd.dma_gather(
                    out_ap=gbuf[:, :kg, :],
                    in_ap=(xtab[:] if half == 0 else xtab_hi),
                    idxs_ap=srcw_sb[:, 8 * t0 : 8 * (t0 + kg)],
                    num_idxs=kg * P,
                    num_idxs_reg=kg * P,
                    elem_size=C,
                    single_packet=SINGLE_PACKET,
                )
                for c0 in range(0, kg, SEL_CHUNK):
                    ck = min(SEL_CHUNK, kg - c0)
                    sel3 = spool.tile([P, SEL_CHUNK, SB], f16, tag="sel")
                    nc.vector.tensor_tensor(
                        out=sel3[:, :ck, :],
                        in0=iota_sb[:, :ck, :],
                        in1=dsts_sb[:, t0 + c0 : t0 + c0 + ck]
                        .unsqueeze(2)
                        .to_broadcast([P, ck, SB]),
                        op=Op.is_equal,
                    )
                    nc.vector.tensor_tensor(
                        out=sel3[:, :ck, :],
                        in0=sel3[:, :ck, :],
                        in1=nrms_sb[:, t0 + c0 : t0 + c0 + ck]
                        .unsqueeze(2)
                        .to_broadcast([P, ck, SB]),
                        op=Op.mult,
                    )
                    for slot in range(c0, c0 + ck):
                        t = t0 + slot
                        bb = int(sb_of[t])
                        if first_of[t]:
                            agg_ps = agg_psp.tile(
                                [C, SB], f32, tag="aggps", name=f"aggps_b{bb}"
                            )
                        nc.tensor.matmul(
                            agg_ps[:],
                            lhsT=gbuf[:, slot, :],
                            rhs=sel3[:, slot - c0, :],
                            start=bool(first_of[t]),
                            stop=bool(last_of[t]),
                        )
                        if last_of[t]:
                            agg_sb = apool.tile([C, SB], f32, tag="aggsb")
                            nc.scalar.activation(agg_sb[:], agg_ps[:], Copy)
                            outT_ps = out_psp.tile([C, SB], f32, tag="outps")
                            nc.tensor.matmul(
                                outT_ps[:],
                                lhsT=wnT_sb[:],
                                rhs=agg_sb[:],
                                start=True,
                                stop=True,
                            )
                            outT_sb = opool.tile([C, SB], f32, tag="outsb")
                            nc.vector.tensor_scalar(
                                out=outT_sb[:],
                                in0=outT_ps[:],
                                scalar1=bias_sb[:],
                                scalar2=None,
                                op0=Op.add,
                            )
                            nc.sync.dma_start(
                                outt[:, bb * SB : (bb + 1) * SB], outT_sb[:]
                            )

    nc.compile()
    return nc


def kernel(x, edge_index, W, b):
    global LAST_RESULTS
    x = np.asarray(x)
    N = x.shape[0]
    assert x.shape[1] == C and W.shape == (C, C)

    in_maps, st = _prep(x, edge_index, W, b)
    nc = _build_program(st)

    os.environ.setdefault("BASS_NEVER_TRACE", "1")
    res = run_bass_kernel_spmd(nc, in_maps, list(range(N_CORES)))
    LAST_RESULTS = res

    npc = st["npc"]
    shards = []
    for s in range(N_CORES):
        lo = s * npc
        hi = min((s + 1) * npc, N)
        outt = res.results[s]["outt"]
        shards.append(outt[:, : hi - lo].T)
    return np.ascontiguousarray(np.concatenate(shards, axis=0), dtype=np.float32)
